# revision 20
# baseline (speedup 1.0000x reference)
"""Block self-attention (chunked, q=k=v, no projections) on 8 Trainium2 cores.

Math (per reference):
  x:[B,S,D] -> [B,H,S,dh] -> chunks of 256 along S -> per (b,chunk,head):
    A = x_chunk  [256, 64]
    S = A @ A.T / 8 + mask      (mask is all-zeros per the input spec)
    P = softmax(S, axis=-1)
    O = P @ A
  -> reassembled to [B,S,D].

Key structural facts used by the kernel:
  * S is symmetric (q=k=v), so the score tiles produced as [q-rows, k-cols]
    can be reused verbatim as the [k-rows, q-cols] stationary operand of the
    second matmul -- no on-chip transpose of the softmax matrix is needed.
  * The softmax denominator is obtained by appending a ones-column to the
    second matmul's moving operand (sum_k E[q,k] * 1).
  * Softmax max-subtraction is replaced by a global constant bias computed
    on the host from the Cauchy-Schwarz bound max_h,i |q_hi|^2 / 8 (~15.9 for
    the spec'd input); exp(x/8 + bias) stays inside fp16 range and the shift
    divides out exactly in the normalization.
  * The device works entirely from a host-precast fp16 copy of the input,
    so no on-chip casts are needed; A^T tiles are built with cheap fp16 PE
    transposes (1 cycle/row).

Sharding: data-parallel over the fused (batch * chunk) dim: 64 chunks total,
8 consecutive chunks per core == one contiguous [2048, 1024] row-slice of the
flattened [16384, 1024] input per core.
"""

import numpy as np

B, S, D = 4, 4096, 1024
H = 16
DH = D // H              # 64
CHUNK = 256
NCORES = 8
ROWS_PER_CORE = (B * S) // NCORES        # 2048
CHUNKS_PER_CORE = ROWS_PER_CORE // CHUNK  # 8
SCALE = 1.0 / 8.0        # 1/sqrt(dh)
# exp() runs as exp(score/8 + bias) with bias calibrated per call so the
# largest value stays inside fp16 range (see kernel()); the shift divides out
# exactly in the softmax normalization.
EXP_MARGIN = 10.5        # ln(65504) ~ 11.09; leave ~0.6 of headroom

_PROGRAM = None


def _build_program():
    import concourse.bass as bass
    import concourse.tile as tile
    from concourse import bacc, mybir
    from concourse.masks import make_identity

    f32 = mybir.dt.float32
    f16 = mybir.dt.float16
    Exp = mybir.ActivationFunctionType.Exp

    nc = bacc.Bacc("TRN2", target_bir_lowering=False, debug=False,
                   num_devices=NCORES)
    xh = nc.dram_tensor("xh", [ROWS_PER_CORE, D], f16, kind="ExternalInput")
    eb = nc.dram_tensor("eb", [128, 1], f32, kind="ExternalInput")
    y = nc.dram_tensor("y", [ROWS_PER_CORE, D], f32, kind="ExternalOutput")
    xhap = xh.ap()
    yap = y.ap()

    GW = DH + 1  # per-head group width in the ones-augmented moving operand

    with tile.TileContext(nc) as tc:
        with (
            tc.tile_pool(name="const", bufs=1) as const_pool,
            tc.tile_pool(name="xb", bufs=6) as xb_pool,
            tc.tile_pool(name="xc", bufs=6) as xc_pool,
            tc.tile_pool(name="xt_ps", bufs=2, space="PSUM") as xtps_pool,
            tc.tile_pool(name="xt_sb", bufs=8) as xtsb_pool,
            tc.tile_pool(name="scores", bufs=2, space="PSUM") as sc_pool,
            tc.tile_pool(name="expv", bufs=8) as e_pool,
            tc.tile_pool(name="outps", bufs=2, space="PSUM") as o_pool,
            tc.tile_pool(name="rcp", bufs=16) as r_pool,
            tc.tile_pool(name="yout", bufs=6) as y_pool,
        ):
            ebias = const_pool.tile([128, 1], f32)
            nc.sync.dma_start(out=ebias[:], in_=eb.ap())
            ident = const_pool.tile([128, 128], f16)
            make_identity(nc, ident[:])

            def emit_front(c, hp, xc):
                # transposes + scores + exp for pair (c, hp); returns the
                # context needed by the back half (mm2 + normalization).
                xt_ps = xtps_pool.tile([128, CHUNK], f16, tag="xtps",
                                       name=f"xtps{c}_{hp}")
                for r in range(2):
                    nc.tensor.transpose(
                        out=xt_ps[:, r * 128:(r + 1) * 128],
                        in_=xc[r][:, hp * 128:(hp + 1) * 128],
                        identity=ident[:],
                    )
                xt = xtsb_pool.tile([128, CHUNK], f16, tag="xt",
                                    name=f"xt{c}_{hp}")
                nc.vector.tensor_copy(out=xt[:], in_=xt_ps[:])

                # Scores for both heads of the pair into one 2-bank PSUM
                # tile: [h0-q0 | h0-q1 | h1-q0 | h1-q1], each [128, 256].
                s_ps = sc_pool.tile([128, 4 * CHUNK], f32, tag="sc",
                                    name=f"sc{c}_{hp}")
                for hi in range(2):
                    for qm in range(2):
                        col = (2 * hi + qm) * CHUNK
                        nc.tensor.matmul(
                            out=s_ps[:, col:col + CHUNK],
                            lhsT=xt[64 * hi:64 * hi + 64,
                                    qm * 128:(qm + 1) * 128],
                            rhs=xt[64 * hi:64 * hi + 64, :],
                            start=True, stop=True,
                        )

                # exp(score/8 + bias) for both heads in one ACT op.
                e_sb = e_pool.tile([128, 4 * CHUNK], f16, tag="e",
                                   name=f"e{c}_{hp}")
                nc.scalar.activation(out=e_sb[:], in_=s_ps[:], func=Exp,
                                     scale=SCALE, bias=ebias[:])
                return e_sb

            def emit_back(c, hp, e_sb, xb, yt, row0):
                # O_unnorm = E @ [A | 1]; symmetry lets the stored score
                # tiles act as the [k, q] stationary operand directly.
                # All 4 (head, q-half) groups of the pair share one PSUM
                # tile so the normalization batches.
                o_ps = o_pool.tile([128, 4 * GW], f32, tag="o",
                                   name=f"o{c}_{hp}")
                for hi in range(2):
                    h = 2 * hp + hi
                    for qm in range(2):
                        g = 2 * hi + qm
                        for r in range(2):
                            base = (2 * hi + r) * CHUNK + qm * 128
                            nc.tensor.matmul(
                                out=o_ps[:, g * GW:(g + 1) * GW],
                                lhsT=e_sb[:, base:base + 128],
                                rhs=xb[r][:, h * GW:(h + 1) * GW],
                                start=(r == 0), stop=(r == 1),
                            )
                rc = r_pool.tile([128, 4], f32, tag="rcp",
                                 name=f"rc{c}_{hp}")
                o_g = o_ps[:].rearrange("p (g c) -> p g c", c=GW)
                nc.vector.reciprocal(
                    out=rc[:].rearrange("p (g c) -> p g c", c=1),
                    in_=o_g[:, :, DH:GW])
                for qm in range(2):
                    # groups {qm, 2+qm} = heads (2hp, 2hp+1) for this
                    # seq-half; one broadcast multiply covers both.
                    out_v = yt[qm][:, hp * 128:(hp + 1) * 128].rearrange(
                        "p (hi c) -> p hi c", hi=2)
                    in0 = bass.AP(tensor=o_ps.tensor,
                                  offset=o_ps.offset + qm * GW,
                                  ap=[o_ps.ap[0], [2 * GW, 2], [1, DH]])
                    in1 = bass.AP(tensor=rc.tensor,
                                  offset=rc.offset + qm,
                                  ap=[rc.ap[0], [2, 2], [0, DH]])
                    nc.vector.tensor_mul(out_v, in0, in1)
                if hp == H // 2 - 1:
                    for r in range(2):
                        nc.sync.dma_start(
                            out=yap[row0 + r * 128: row0 + (r + 1) * 128, :],
                            in_=yt[r][:])

            # One-pair software pipeline: the front half (transposes, scores,
            # exp) of pair p+1 is emitted BEFORE the back half (PV matmul,
            # normalization) of pair p, so the scheduler keeps the ACT engine
            # (the bottleneck) fed ahead of PE's second-matmul work.
            pending = None
            for c in range(CHUNKS_PER_CORE):
                row0 = c * CHUNK

                # Chunk load (fp16, contiguous), then the PV moving operand:
                # per head [A_h | 1] groups of 65 columns, built on GpSimd.
                xc = []
                for r in range(2):
                    t = xc_pool.tile([128, D], f16, tag="xc",
                                     name=f"xc{c}_{r}")
                    rows = xhap[row0 + r * 128: row0 + (r + 1) * 128, :]
                    if c == 0:
                        # First chunk: land the first head-pairs' columns
                        # early so the PE/ACT pipeline fills sooner.
                        nc.sync.dma_start(out=t[:, 0:256], in_=rows[:, 0:256])
                        nc.sync.dma_start(out=t[:, 256:D], in_=rows[:, 256:D])
                    else:
                        nc.sync.dma_start(out=t[:], in_=rows)
                    xc.append(t)

                xb = []
                for r in range(2):
                    t = xb_pool.tile([128, H * GW], f16, tag="xb",
                                     name=f"xb{c}_{r}")
                    dst = t[:].rearrange("p (g c) -> p g c", c=GW)
                    nc.gpsimd.tensor_copy(
                        out=dst[:, :, 0:DH],
                        in_=xc[r][:].rearrange("p (g c) -> p g c", c=DH))
                    nc.gpsimd.memset(dst[:, :, DH:GW], 1.0)
                    xb.append(t)

                yt = [y_pool.tile([128, D], f32, tag="yout", name=f"yt{c}_{r}")
                      for r in range(2)]

                for hp in range(H // 2):
                    e_sb = emit_front(c, hp, xc)
                    if pending is not None:
                        emit_back(*pending)
                    pending = (c, hp, e_sb, xb, yt, row0)
            emit_back(*pending)

    nc.compile()
    return nc


def _get_program():
    global _PROGRAM
    if _PROGRAM is None:
        _PROGRAM = _build_program()
    return _PROGRAM


def _reference_numpy(hs, mask):
    # Exact reference math in numpy; only used if a nonzero mask ever shows up
    # (the input spec pins the mask to zeros).
    NC_ = S // CHUNK
    xx = hs.reshape(B, S, H, DH).transpose(0, 2, 1, 3)
    q = xx.reshape(B * NC_, H, CHUNK, DH)
    m = mask.reshape(B * NC_, 1, 1, CHUNK)
    scores = np.einsum('bhqd,bhkd->bhqk', q, q) / np.sqrt(DH) + m
    scores -= scores.max(axis=-1, keepdims=True)
    probs = np.exp(scores)
    probs /= probs.sum(axis=-1, keepdims=True)
    ctx = np.einsum('bhqk,bhkd->bhqd', probs, q)
    return ctx.reshape(B, H, S, DH).transpose(0, 2, 1, 3).reshape(B, S, D).astype(np.float32)


def _run(flat16, exp_bias=-5.5, trace=False, trace_kwargs=None):
    from concourse.bass_utils import run_bass_kernel_spmd
    nc = _get_program()
    ebv = np.full((128, 1), exp_bias, dtype=np.float32)
    in_maps = [{"xh": np.ascontiguousarray(
        flat16[i * ROWS_PER_CORE:(i + 1) * ROWS_PER_CORE]),
        "eb": ebv}
        for i in range(NCORES)]
    return run_bass_kernel_spmd(nc, in_maps, core_ids=list(range(NCORES)),
                                trace=trace, **(trace_kwargs or {}))


def kernel(hidden_states, attention_mask):
    hs = np.ascontiguousarray(np.asarray(hidden_states, dtype=np.float32))
    mask = np.asarray(attention_mask, dtype=np.float32)
    assert hs.shape == (B, S, D)
    if mask.size and np.any(mask != 0.0):
        return _reference_numpy(hs, mask)
    flat16 = hs.reshape(B * S, D).astype(np.float16)
    # Cauchy-Schwarz: max score <= max_h,i |q_hi|^2; pick the exp shift so the
    # largest exp() input is ~EXP_MARGIN (fits fp16 with headroom).
    max_scaled = float((flat16.astype(np.float32) ** 2)
                       .reshape(-1, H, DH).sum(-1).max()) * SCALE
    exp_bias = min(EXP_MARGIN - max_scaled, 0.0)
    res = _run(flat16, exp_bias=exp_bias)
    out = np.concatenate([res.results[i]["y"] for i in range(NCORES)], axis=0)
    return out.reshape(B, S, D).astype(np.float32)
